# revision 1
# baseline (speedup 1.0000x reference)
"""Trainium2 Bass kernel for nn_Block_13391708030014 (dense transformer block).

Sharding: data-parallel over batch — core b computes batch item b entirely
(B == n_cores == 8), no collectives.

Per-core structure:
  A. ln1 token-major (bn_stats; LN affine folded into downstream weights on
     the host), PE-transpose to feature-major hcT; q = qw^T @ hcT matmuls
     interleaved per 4-token-tile chunk.
  B. spatial-reduction conv as 16-tap accumulated matmuls on strided views
     of hcT; srn layernorm (transpose/stats/normalize/transpose); k^T,
     v (token-major), va = alpha*v; kbd = block-diagonal head-pair layout
     of k^T so QK yields two heads' scores per N=512 matmul.
  C. attention per (head-group, 4-token-tile group): one [128,512] PSUM
     tile accumulates pos@va (posT is host-transposed + bf16-cast, matmuls
     run straight off the DMA'd tile) and exp(s)@v. Softmax skips
     max-subtraction (logits are tiny): one Exp with accum_out gives
     exp(s) and row-sums; (1-alpha)/sum scaling on GpSimd/DVE; exp(s)
     tiles PE-transpose into a [128,2,512] PSUM tile, one eviction per
     head, then N=512 attn@v matmuls. proj + residual follow per token
     tile, with ln2 + h2T transposes folded in (they hide under C's PE
     work).
  D. fc1 into a zero-padded 66x66 fp8 spatial layout (three planes:
     original, shifted +1, shifted +66); depthwise 3x3 conv as fp8
     DoubleRow diag-matmuls (tap pairs (0,1),(3,4),(6,7),(2,5) + single
     tap 8); bias+Gelu fused into the PSUM eviction; fc2 as fp8 DoubleRow
     over hidden-block pairs accumulated in PSUM, added into the fp32
     residual in SBUF, DMA'd out.

Matmuls are bf16 except the depthwise conv and fc2 (fp8 DoubleRow); the
residual stream stays fp32. Hardware-measured rel err ~7e-4.
"""

from contextlib import ExitStack

import numpy as np
import ml_dtypes

import concourse.bass as bass
import concourse.tile as tile
from concourse import mybir
from concourse.bass_utils import run_bass_kernel_spmd
from concourse.masks import make_identity

F32 = mybir.dt.float32
BF16 = mybir.dt.bfloat16
FP8 = mybir.dt.float8e4
AF = mybir.ActivationFunctionType
OP = mybir.AluOpType
DR = mybir.MatmulPerfMode.DoubleRow

B, N, C = 8, 4096, 256
H, DH = 8, 32
NK = 256
HID = 1024
HW = 64
SR = 4
P = 128
TT = N // P          # 32 token tiles
KB = C // P          # 2 channel blocks
MB = HID // P        # 8 hidden blocks
PADW = HW + 2        # 66
NPAD = PADW * PADW   # 4356
NPAD_AL = 4368       # NPAD padded to a 16-multiple (DoubleRow pair step)


def _split_drain_waits(nc, max_waits=1):
    """walrus in this toolchain refuses instructions with more than one sem
    wait; hoist every wait of a multi-wait instruction onto dedicated
    single-wait NOPs inserted just before it on the same engine (semantically
    identical: same engine, same program order)."""
    for f in nc.m.functions:
        for blk in f.blocks:
            insts = blk.instructions
            new = []
            changed = False
            for inst in insts:
                si = getattr(inst, "sync_info", None)
                if si is not None and si.on_wait and len(si.on_wait) > max_waits:
                    for i, w in enumerate(list(si.on_wait)):
                        new.append(mybir.InstNoOp(
                            name=f"{inst.name}-ws{i}",
                            sync_info=mybir.SyncInfo(on_wait=[w], on_update=[]),
                            bass_nofuse=True,
                            engine=inst.engine,
                        ))
                    inst.sync_info = mybir.SyncInfo(
                        on_wait=[], on_update=list(si.on_update or []))
                    changed = True
                new.append(inst)
            if changed:
                blk.instructions = new


def _bf(x):
    return np.ascontiguousarray(x.astype(ml_dtypes.bfloat16))


def _f8(x):
    return np.ascontiguousarray(x.astype(ml_dtypes.float8_e4m3))


def _prep_weights(i):
    """Fold LN affines into downstream weights; return DRAM payloads."""
    ln1_w, ln1_b = i["ln1_w"], i["ln1_b"]
    srn_w, srn_b = i["srn_w"], i["srn_b"]
    ln2_w, ln2_b = i["ln2_w"], i["ln2_b"]

    qw = ln1_w[:, None] * i["q_w"]                      # [C, C]
    qb = ln1_b @ i["q_w"] + i["q_b"]                    # [C]

    # sr_w is OIHW: [c_out, c_in, dy, dx] -> srw[tap, ci, co]
    srw = (i["sr_w"] * ln1_w[None, :, None, None]).transpose(2, 3, 1, 0)
    srw = np.ascontiguousarray(srw.reshape(SR * SR, C, C))
    srb = i["sr_b"] + np.einsum("i,oihw->o", ln1_b, i["sr_w"])

    kvw = srn_w[:, None] * i["kv_w"]                    # [C, 2C]
    kvb = srn_b @ i["kv_w"] + i["kv_b"]
    kw, vw = kvw[:, :C], kvw[:, C:]
    kb_, vb = kvb[:C], kvb[C:]

    f1w = ln2_w[:, None] * i["fc1_w"]                   # [C, HID]
    f1b = ln2_b @ i["fc1_w"] + i["fc1_b"]

    # depthwise conv -> fp8 block-diag matrices. DoubleRow pairs taps whose
    # padded-layout offsets differ by +1 ((0,1),(3,4),(6,7)) or +66 ((2,5));
    # tap 8 runs as a normal fp8 matmul.
    dww = i["dw_w"].reshape(HID, 9)                     # [HID, tap]
    idx = np.arange(P)
    dwdp = np.zeros((4, MB, P, 2, P), np.float32)
    dwds = np.zeros((MB, P, P), np.float32)
    for j, (ta, tb) in enumerate(((0, 1), (3, 4), (6, 7), (2, 5))):
        for mb in range(MB):
            dwdp[j, mb, idx, 0, idx] = dww[mb * P:(mb + 1) * P, ta]
            dwdp[j, mb, idx, 1, idx] = dww[mb * P:(mb + 1) * P, tb]
    for mb in range(MB):
        dwds[mb, idx, idx] = dww[mb * P:(mb + 1) * P, 8]

    # fc2 as fp8 DoubleRow over hidden-block pairs: [4, 128, 2, C]
    f2w8 = np.ascontiguousarray(
        i["fc2_w"].reshape(MB // 2, 2, P, C).transpose(0, 2, 1, 3))

    return {
        "qw": _bf(qw), "qb": qb.astype(np.float32),
        "srw": _bf(srw), "srb": srb.astype(np.float32),
        "kw": _bf(kw), "kb": kb_.astype(np.float32),
        "vw": _bf(vw), "vb": vb.astype(np.float32),
        "pjw": _bf(i["proj_w"]), "pjb": i["proj_b"].astype(np.float32),
        "f1w": _bf(f1w), "f1b": f1b.astype(np.float32),
        "dwdp": _f8(dwdp), "dwds": _f8(dwds),
        "dwb": i["dw_b"].astype(np.float32),
        "f2w8": _f8(f2w8), "f2b": i["fc2_b"].astype(np.float32),
    }


def _build_program(a, nz):
    nc = bass.Bass("TRN2", target_bir_lowering=False, debug=False,
                   num_devices=B)

    x_d = nc.dram_tensor("x", [N, C], F32, kind="ExternalInput").ap()
    # pos_2D, host-transposed per head to [NK, N] and cast to bf16
    post_d = nc.dram_tensor("post", [H, NK, N], BF16, kind="ExternalInput").ap()
    out_d = nc.dram_tensor("out", [N, C], F32, kind="ExternalOutput").ap()

    w_d = {}
    wshapes = {
        "qw": ([C, C], BF16), "srw": ([16, C, C], BF16),
        "kw": ([C, C], BF16), "vw": ([C, C], BF16),
        "pjw": ([C, C], BF16), "f1w": ([C, HID], BF16),
        "dwdp": ([4, MB, P, 2, P], FP8), "dwds": ([MB, P, P], FP8),
        "dwb": ([HID], F32), "f2w8": ([MB // 2, P, 2, C], FP8),
    }
    for nm in ("qb", "srb", "kb", "vb", "pjb", "f1b", "f2b"):
        if nz[nm]:
            wshapes[nm] = ([{"f1b": HID}.get(nm, C)], F32)
    for nm, (shp, dt) in wshapes.items():
        w_d[nm] = nc.dram_tensor(nm, shp, dt, kind="ExternalInput").ap()

    scale = DH ** -0.5

    with tile.TileContext(nc) as tc, ExitStack() as ctx:
        persist = ctx.enter_context(tc.tile_pool(name="persist", bufs=1))
        wpool = ctx.enter_context(tc.tile_pool(name="weights", bufs=1))
        stat = ctx.enter_context(tc.tile_pool(name="stat", bufs=8))

        # ---- persistent tiles
        hcT = [persist.tile([P, N], BF16, tag=f"hcT{k}", name=f"hcT{k}")
               for k in range(KB)]
        qT = [persist.tile([P, N], BF16, tag=f"qT{k}", name=f"qT{k}")
              for k in range(KB)]
        kT = [persist.tile([P, NK], BF16, tag=f"kT{k}", name=f"kT{k}")
              for k in range(KB)]
        kbd = [persist.tile([P, 2, 512], BF16, tag=f"kbd{g}", name=f"kbd{g}")
               for g in range(KB)]
        vtok = [persist.tile([P, C], BF16, tag=f"vtok{k}", name=f"vtok{k}")
                for k in range(KB)]
        va = [persist.tile([P, C], BF16, tag=f"va{k}", name=f"va{k}")
              for k in range(KB)]
        x2 = persist.tile([P, TT, C], F32, tag="x2")
        h2T = [persist.tile([P, N], BF16, tag=f"h2T{k}", name=f"h2T{k}")
               for k in range(KB)]

        # ---- constants / weights to SBUF
        ident = wpool.tile([P, P], BF16)
        make_identity(nc, ident[:])
        eps1 = wpool.tile([P, 1], F32)
        nc.vector.memset(eps1[:], 1e-6)
        epss = wpool.tile([P, 1], F32)
        nc.vector.memset(epss[:], 1e-5)

        qw_sb = wpool.tile([P, KB, C], BF16)
        nc.sync.dma_start(qw_sb[:], w_d["qw"].rearrange("(k p) c -> p k c", p=P))
        srw_sb = wpool.tile([P, 16, KB, C], BF16)
        nc.sync.dma_start(srw_sb[:],
                          w_d["srw"].rearrange("t (k p) c -> p t k c", p=P))
        kw_sb = wpool.tile([P, KB, C], BF16)
        nc.sync.dma_start(kw_sb[:], w_d["kw"].rearrange("(k p) c -> p k c", p=P))
        vw_sb = wpool.tile([P, KB, C], BF16)
        nc.sync.dma_start(vw_sb[:], w_d["vw"].rearrange("(k p) c -> p k c", p=P))
        pjw_sb = wpool.tile([P, KB, C], BF16)
        nc.sync.dma_start(pjw_sb[:], w_d["pjw"].rearrange("(k p) c -> p k c", p=P))
        f1w_sb = wpool.tile([P, KB, HID], BF16)
        nc.sync.dma_start(f1w_sb[:], w_d["f1w"].rearrange("(k p) c -> p k c", p=P))
        f2w_sb = wpool.tile([P, MB // 2, 2, C], FP8)
        nc.sync.dma_start(f2w_sb[:],
                          w_d["f2w8"].rearrange("g p two c -> p g two c"))
        dwb_sb = wpool.tile([P, MB], F32)
        nc.sync.dma_start(dwb_sb[:], w_d["dwb"].rearrange("(m p) -> p m", p=P))

        bias_sb = {}
        for nm, dim in (("qb", C), ("srb", C), ("kb", C), ("f1b", HID)):
            if nz[nm]:
                t = wpool.tile([P, dim // P], F32, name=f"bias_{nm}")
                nc.sync.dma_start(t[:], w_d[nm].rearrange("(k p) -> p k", p=P))
                bias_sb[nm] = t
        for nm in ("vb", "pjb", "f2b"):
            if nz[nm]:  # free-axis bias: broadcast across partitions
                t = wpool.tile([P, C], F32, name=f"biasbc_{nm}")
                nc.sync.dma_start(t[:], w_d[nm].to_broadcast([P, C]))
                bias_sb[nm] = t

        def ln_norm(src_ap, eps_tile, out_tile):
            """token-major LN core: out = (src - mean) * rsqrt(var + eps)."""
            st = stat.tile([P, 6], F32, tag="st", name="st")
            nc.vector.bn_stats(out=st[:], in_=src_ap)
            mv = stat.tile([P, 2], F32, tag="mv", name="mv")
            nc.vector.bn_aggr(out=mv[:], in_=st[:])
            rs = stat.tile([P, 1], F32, tag="rs", name="rs")
            nc.scalar.activation(rs[:], mv[:, 1:2], AF.Sqrt, bias=eps_tile[:])
            nc.vector.reciprocal(rs[:], rs[:])
            nc.vector.tensor_scalar(
                out=out_tile[:], in0=src_ap, scalar1=mv[:, 0:1], scalar2=rs[:],
                op0=OP.subtract, op1=OP.mult)

        # ========== phase A: ln1 + transpose + q =====================
        with ExitStack() as pctx:
            xpool = pctx.enter_context(tc.tile_pool(name="xa", bufs=3))
            hcpool = pctx.enter_context(tc.tile_pool(name="hca", bufs=4))
            tpA = pctx.enter_context(
                tc.tile_pool(name="tpA", bufs=4, space="PSUM"))
            qa_ps = pctx.enter_context(
                tc.tile_pool(name="qaps", bufs=2, space="PSUM"))
            x4 = None
            for tt in range(TT):
                if tt % 4 == 0:
                    x4 = xpool.tile([P, 4, C], F32, tag="x4", name="x4")
                    nc.sync.dma_start(
                        x4[:], x_d.rearrange("(g q p) c -> g p q c", p=P,
                                             q=4)[tt // 4])
                hc = hcpool.tile([P, C], BF16, name="hc")
                ln_norm(x4[:, tt % 4, :], eps1, hc)
                for kb in range(KB):
                    pt = tpA.tile([P, P], BF16, name="ptA")
                    nc.tensor.transpose(
                        pt[:], hc[:, kb * P:(kb + 1) * P], ident[:])
                    nc.scalar.copy(
                        out=hcT[kb][:, tt * P:(tt + 1) * P], in_=pt[:])
                if tt % 4 == 3:
                    nt = tt // 4
                    for cb in range(KB):
                        ps = qa_ps.tile([P, 512], F32, name="qps")
                        for kb in range(KB):
                            nc.tensor.matmul(
                                ps[:], qw_sb[:, kb, cb * P:(cb + 1) * P],
                                hcT[kb][:, nt * 512:(nt + 1) * 512],
                                start=(kb == 0), stop=(kb == KB - 1))
                        dst = qT[cb][:, nt * 512:(nt + 1) * 512]
                        if nz["qb"]:
                            nc.vector.tensor_scalar(
                                out=dst, in0=ps[:],
                                scalar1=bias_sb["qb"][:, cb:cb + 1],
                                scalar2=None, op0=OP.add)
                        else:
                            nc.vector.tensor_copy(out=dst, in_=ps[:])

        # ========== phase B: SR-conv, srn, k, v ======================
        with ExitStack() as pctx:
            mm_ps = pctx.enter_context(
                tc.tile_pool(name="mmB", bufs=3, space="PSUM"))
            tpB = pctx.enter_context(
                tc.tile_pool(name="tpB", bufs=4, space="PSUM"))
            bwork = pctx.enter_context(tc.tile_pool(name="bwork", bufs=1))

            # SR conv -> hsT (feature-major [co, nk])
            hsT = [bwork.tile([P, NK], BF16, tag=f"hsT{c}", name=f"hsT{c}")
                   for c in range(KB)]
            conv_rhs = [
                hcT[kb].rearrange("p (r a c b) -> p a b r c", a=SR, b=SR,
                                  c=HW // SR)
                for kb in range(KB)]
            for cob in range(KB):
                ps = mm_ps.tile([P, NK], F32, tag="mm", name="psconv")
                first = True
                for tap in range(16):
                    dy, dx = tap // SR, tap % SR
                    for kb in range(KB):
                        nc.tensor.matmul(
                            ps[:], srw_sb[:, tap, kb, cob * P:(cob + 1) * P],
                            conv_rhs[kb][:, dy, dx, :, :],
                            start=first, stop=(tap == 15 and kb == KB - 1))
                        first = False
                if nz["srb"]:
                    nc.vector.tensor_scalar(
                        out=hsT[cob][:], in0=ps[:],
                        scalar1=bias_sb["srb"][:, cob:cob + 1],
                        scalar2=None, op0=OP.add)
                else:
                    nc.vector.tensor_copy(out=hsT[cob][:], in_=ps[:])

            # srn layernorm (transpose -> stats -> normalize -> transpose)
            hs_tok = [bwork.tile([P, C], BF16, tag=f"hstok{k}",
                                 name=f"hstok{k}") for k in range(KB)]
            for nkb in range(KB):
                for cb in range(KB):
                    pt = tpB.tile([P, P], BF16, tag="ptB", name="ptB")
                    nc.tensor.transpose(
                        pt[:], hsT[cb][:, nkb * P:(nkb + 1) * P], ident[:])
                    nc.vector.tensor_copy(
                        out=hs_tok[nkb][:, cb * P:(cb + 1) * P], in_=pt[:])
            hsnT = [bwork.tile([P, NK], BF16, tag=f"hsnT{k}", name=f"hsnT{k}")
                    for k in range(KB)]
            for nkb in range(KB):
                hsn = bwork.tile([P, C], BF16, tag=f"hsn{nkb}",
                                 name=f"hsn{nkb}")
                ln_norm(hs_tok[nkb][:], epss, hsn)
                for cb in range(KB):
                    pt = tpB.tile([P, P], BF16, tag="ptB", name="ptB2")
                    nc.tensor.transpose(
                        pt[:], hsn[:, cb * P:(cb + 1) * P], ident[:])
                    nc.vector.tensor_copy(
                        out=hsnT[cb][:, nkb * P:(nkb + 1) * P], in_=pt[:])

            # k^T [c, nk]
            for cb in range(KB):
                ps = mm_ps.tile([P, NK], F32, tag="mm", name="psk")
                for kb in range(KB):
                    nc.tensor.matmul(
                        ps[:], kw_sb[:, kb, cb * P:(cb + 1) * P], hsnT[kb][:],
                        start=(kb == 0), stop=(kb == KB - 1))
                if nz["kb"]:
                    nc.vector.tensor_scalar(
                        out=kT[cb][:], in0=ps[:],
                        scalar1=bias_sb["kb"][:, cb:cb + 1],
                        scalar2=None, op0=OP.add)
                else:
                    nc.vector.tensor_copy(out=kT[cb][:], in_=ps[:])
            # block-diag head-pair layout for batched QK
            for hg in range(KB):
                nc.vector.memset(kbd[hg][:], 0.0)
                for hh in range(4):
                    j, half = hh // 2, hh % 2
                    nc.vector.tensor_copy(
                        out=kbd[hg][hh * 32:(hh + 1) * 32, j,
                                    half * 256:(half + 1) * 256],
                        in_=kT[hg][hh * 32:(hh + 1) * 32, :])
            # v token-major [nk, c]; va = alpha * v for the pos path
            for nkb in range(KB):
                ps = mm_ps.tile([P, C], F32, tag="mm", name="psv")
                for kb in range(KB):
                    nc.tensor.matmul(
                        ps[:], hsnT[kb][:, nkb * P:(nkb + 1) * P],
                        vw_sb[:, kb, :],
                        start=(kb == 0), stop=(kb == KB - 1))
                nc.vector.tensor_copy(out=vtok[nkb][:], in_=ps[:])
                if nz["vb"]:
                    nc.vector.tensor_add(
                        out=vtok[nkb][:], in0=vtok[nkb][:], in1=bias_sb["vb"][:])
                nc.vector.tensor_scalar(
                    out=va[nkb][:], in0=vtok[nkb][:], scalar1=a,
                    scalar2=None, op0=OP.mult)

        # ========== phase C: attention (+ ln2/h2T folded in) ==========
        with ExitStack() as pctx:
            xpool = pctx.enter_context(tc.tile_pool(name="xc", bufs=2))
            pospool = pctx.enter_context(tc.tile_pool(name="pos", bufs=3))
            epool = pctx.enter_context(tc.tile_pool(name="eatt", bufs=6))
            espool = pctx.enter_context(tc.tile_pool(name="esatt", bufs=20))
            estp = pctx.enter_context(tc.tile_pool(name="estp", bufs=3))
            otpool = pctx.enter_context(tc.tile_pool(name="otp", bufs=2))
            h2cpool = pctx.enter_context(tc.tile_pool(name="h2cc", bufs=3))
            s_ps = pctx.enter_context(
                tc.tile_pool(name="sps", bufs=2, space="PSUM"))
            et_ps = pctx.enter_context(
                tc.tile_pool(name="etps", bufs=2, space="PSUM"))
            o_ps = pctx.enter_context(
                tc.tile_pool(name="ops", bufs=2, space="PSUM"))
            pj_ps = pctx.enter_context(
                tc.tile_pool(name="pjps", bufs=1, space="PSUM"))

            for ttg in range(8):
                oTs = []
                for hg in range(KB):
                    op_t = o_ps.tile([P, 512], F32, name="opt")
                    # pos path: one batched DMA (4 heads) per nkb, then
                    # 8 matmuls at N=512
                    for nkb in range(KB):
                        pos_sb = pospool.tile([P, 4, 512], BF16, name="possb")
                        nc.sync.dma_start(
                            pos_sb[:],
                            post_d.rearrange(
                                "(g hh) nk n -> g nk hh n", g=KB)[
                                hg, nkb * P:(nkb + 1) * P, :,
                                ttg * 512:(ttg + 1) * 512])
                        for hh in range(4):
                            h = hg * 4 + hh
                            nc.tensor.matmul(
                                op_t[hh * 32:(hh + 1) * 32, :],
                                va[nkb][:, h * 32:(h + 1) * 32],
                                pos_sb[:, hh, :],
                                start=(nkb == 0), stop=False,
                                tile_position=(0, hh * 32))
                    # exp(s) path: QK via block-diag head pairs; es
                    # collected for 4 token tiles, then per head one
                    # [128,2,512] transpose PSUM + single eviction + two
                    # N=512 attn@v matmuls
                    es_all = {}
                    for t4 in range(4):
                        tt = ttg * 4 + t4
                        for j in range(2):
                            sps = s_ps.tile([P, 512], F32, name="sps")
                            nc.tensor.matmul(
                                sps[:], qT[hg][:, tt * P:(tt + 1) * P],
                                kbd[hg][:, j, :], start=True, stop=True)
                            for half in range(2):
                                hh = j * 2 + half
                                e = epool.tile([P, NK], F32, name="e")
                                ssum = stat.tile([P, 1], F32, tag="ssum",
                                                 name="ssum")
                                nc.scalar.activation(
                                    e[:], sps[:, half * 256:(half + 1) * 256],
                                    AF.Exp, scale=scale, accum_out=ssum[:])
                                f = stat.tile([P, 1], F32, tag="f", name="f")
                                nc.vector.reciprocal(f[:], ssum[:])
                                es = espool.tile([P, NK], BF16, tag="es",
                                                 name=f"es{t4}_{hh}")
                                eng = nc.vector if hh == 0 else nc.gpsimd
                                eng.tensor_scalar(
                                    out=es[:], in0=e[:], scalar1=f[:],
                                    scalar2=1.0 - a, op0=OP.mult, op1=OP.mult)
                                es_all[(t4, hh)] = es
                    for hh in range(4):
                        h = hg * 4 + hh
                        et = et_ps.tile([P, 2, 512], BF16, name="et")
                        for t4 in range(4):
                            for nkb in range(KB):
                                nc.tensor.transpose(
                                    et[:, nkb, t4 * P:(t4 + 1) * P],
                                    es_all[(t4, hh)][:, nkb * P:(nkb + 1) * P],
                                    ident[:])
                        etsb = estp.tile([P, 2, 512], BF16, name="etsb")
                        if hh % 2 == 0:
                            nc.vector.tensor_copy(out=etsb[:], in_=et[:])
                        else:
                            nc.scalar.copy(out=etsb[:], in_=et[:])
                        for nkb in range(KB):
                            nc.tensor.matmul(
                                op_t[hh * 32:(hh + 1) * 32, :],
                                vtok[nkb][:, h * 32:(h + 1) * 32],
                                etsb[:, nkb, :],
                                start=False, stop=(nkb == KB - 1),
                                tile_position=(0, hh * 32))
                    ot = otpool.tile([P, 512], BF16, tag=f"oTs{hg}",
                                     name=f"oTs{hg}")
                    nc.vector.tensor_copy(out=ot[:], in_=op_t[:])
                    oTs.append(ot)
                # proj + residual + ln2/h2T for the 4 token tiles
                x4 = xpool.tile([P, 4, C], F32, name="x4c")
                nc.sync.dma_start(
                    x4[:],
                    x_d.rearrange("(g q p) c -> g p q c", p=P, q=4)[ttg])
                for t4 in range(4):
                    tt = ttg * 4 + t4
                    pps = pj_ps.tile([P, C], F32, tag="pps", name="pps", bufs=1)
                    for hg in range(KB):
                        nc.tensor.matmul(
                            pps[:], oTs[hg][:, t4 * P:(t4 + 1) * P],
                            pjw_sb[:, hg, :],
                            start=(hg == 0), stop=(hg == KB - 1))
                    if nz["pjb"]:
                        nc.vector.tensor_add(
                            out=x2[:, tt, :], in0=pps[:], in1=bias_sb["pjb"][:])
                        nc.vector.tensor_add(
                            out=x2[:, tt, :], in0=x2[:, tt, :],
                            in1=x4[:, t4, :])
                    else:
                        nc.vector.tensor_tensor(
                            out=x2[:, tt, :], in0=x4[:, t4, :], in1=pps[:],
                            op=OP.add)
                    # ln2 + h2T (hides under C's attention PE work)
                    h2c = h2cpool.tile([P, C], BF16, name="h2c")
                    ln_norm(x2[:, tt, :], eps1, h2c)
                    for kb in range(KB):
                        pt = pj_ps.tile([P, P], BF16, tag="tpC", name="ptC", bufs=1)
                        nc.tensor.transpose(
                            pt[:], h2c[:, kb * P:(kb + 1) * P], ident[:])
                        if kb == 0:
                            nc.scalar.copy(
                                out=h2T[kb][:, tt * P:(tt + 1) * P], in_=pt[:])
                        else:
                            nc.vector.tensor_copy(
                                out=h2T[kb][:, tt * P:(tt + 1) * P], in_=pt[:])

        # ========== phase D: MLP =====================================
        with ExitStack() as pctx:
            mpadp = pctx.enter_context(tc.tile_pool(name="mpad", bufs=2))
            m2cp = pctx.enter_context(tc.tile_pool(name="m2c", bufs=2))
            dwdp = pctx.enter_context(tc.tile_pool(name="dwd", bufs=2))
            mm_ps = pctx.enter_context(
                tc.tile_pool(name="mmD", bufs=4, space="PSUM"))
            f2_ps = pctx.enter_context(
                tc.tile_pool(name="f2ps", bufs=2, space="PSUM"))

            for mbq in (0, 4):
                m2pairs = []
                for mb in range(mbq, mbq + 4):
                    # fc1 -> padded fp8 layout (plane 0)
                    mpad = mpadp.tile([P, 3, NPAD_AL], FP8, tag="mpad",
                                      name=f"mpad{mb}")
                    vp = mpad[:, 0, 0:NPAD].rearrange(
                        "p (r c) -> p r c", c=PADW)
                    vpq = mpad[:, 0:2, 0:NPAD].rearrange(
                        "p q (r c) -> p q r c", c=PADW)
                    vpq2 = mpad[:, 0:3:2, 0:NPAD].rearrange(
                        "p q (r c) -> p q r c", c=PADW)
                    nc.gpsimd.memset(vp[:, 0, :], 0.0)
                    nc.gpsimd.memset(vp[:, PADW - 1, :], 0.0)
                    nc.gpsimd.memset(vp[:, 1:PADW - 1, 0:1], 0.0)
                    nc.gpsimd.memset(vp[:, 1:PADW - 1, PADW - 1:PADW], 0.0)
                    for nt in range(8):
                        ps = mm_ps.tile([P, 512], F32, tag="mmd", name="psf1")
                        for kb in range(KB):
                            nc.tensor.matmul(
                                ps[:], f1w_sb[:, kb, mb * P:(mb + 1) * P],
                                h2T[kb][:, nt * 512:(nt + 1) * 512],
                                start=(kb == 0), stop=(kb == KB - 1))
                        dst = vp[:, 1 + 8 * nt:1 + 8 * nt + 8, 1:65]
                        src = ps.rearrange("p (r c) -> p r c", c=HW)
                        if nz["f1b"]:
                            eng = nc.vector if nt % 2 else nc.scalar
                            eng.tensor_scalar(
                                out=dst, in0=src,
                                scalar1=bias_sb["f1b"][:, mb:mb + 1],
                                scalar2=None, op0=OP.add)
                        elif nt % 2:
                            nc.vector.tensor_copy(out=dst, in_=src)
                        else:
                            nc.scalar.activation(dst, src, AF.Copy, bias=0.0)
                    # planes 1/2 = plane 0 shifted by +1 / +66 elements, so
                    # a DoubleRow pair reads both taps at one offset
                    nc.sync.dma_start(
                        out=mpad[:, 1, 0:NPAD - 1], in_=mpad[:, 0, 1:NPAD])
                    nc.sync.dma_start(
                        out=mpad[:, 2, 0:NPAD - PADW],
                        in_=mpad[:, 0, PADW:NPAD])
                    # depthwise conv: 4 fp8 DoubleRow pairs + 1 single
                    dwp_sb = dwdp.tile([P, 4, 2, P], FP8, tag="dwdp",
                                       name=f"dwp{mb}")
                    nc.sync.dma_start(
                        dwp_sb[:],
                        w_d["dwdp"][:, mb].rearrange("j q two c -> q j two c"))
                    dws_sb = dwdp.tile([P, P], FP8, tag="dwds",
                                       name=f"dws{mb}")
                    nc.sync.dma_start(
                        dws_sb[:], w_d["dwds"][mb].rearrange("q c -> q c"))
                    if mb % 2 == 0:
                        m2pair = m2cp.tile([P, 2, N], FP8, tag="m2c",
                                           name=f"m2pair{mb}")
                        m2pairs.append(m2pair)
                    m2c = m2pair[:, mb % 2, :]
                    for rb in range(8):
                        dps = mm_ps.tile([P, 512], F32, tag="mmd", name="psdw")
                        for j in range(3):   # pairs (0,1),(3,4),(6,7): dy=j
                            rhs = vpq[:, :, 8 * rb + j:8 * rb + j + 8, 0:HW]
                            nc.tensor.matmul(
                                dps[:], dwp_sb[:, j, :, :], rhs,
                                start=(j == 0), stop=False, perf_mode=DR)
                        # pair (2,5): tap2=(0,2) plane0, tap5=(1,2)=+66
                        rhs = vpq2[:, :, 8 * rb:8 * rb + 8, 2:2 + HW]
                        nc.tensor.matmul(
                            dps[:], dwp_sb[:, 3, :, :], rhs,
                            start=False, stop=False, perf_mode=DR)
                        # single tap 8 = (2,2)
                        rhs = vp[:, 8 * rb + 2:8 * rb + 2 + 8, 2:2 + HW]
                        nc.tensor.matmul(
                            dps[:], dws_sb[:], rhs, start=False, stop=True)
                        nc.scalar.activation(
                            m2c[:, rb * 512:(rb + 1) * 512], dps[:], AF.Gelu,
                            bias=dwb_sb[:, mb:mb + 1])
                # fc2: fp8 DoubleRow over hidden-block pairs
                for tt in range(TT):
                    fps = f2_ps.tile([P, C], F32, name="fps")
                    for j in range(2):
                        nc.tensor.matmul(
                            fps[:], m2pairs[j][:, :, tt * P:(tt + 1) * P],
                            f2w_sb[:, mbq // 2 + j, :, :],
                            start=(j == 0), stop=(j == 1), perf_mode=DR)
                    nc.vector.tensor_tensor(
                        out=x2[:, tt, :], in0=x2[:, tt, :], in1=fps[:],
                        op=OP.add)

            if nz["f2b"]:
                for tt in range(TT):
                    nc.vector.tensor_add(
                        out=x2[:, tt, :], in0=x2[:, tt, :],
                        in1=bias_sb["f2b"][:])

            outr = out_d.rearrange("(g q p) c -> g p q c", p=P, q=4)
            for g in range(TT // 4):
                nc.sync.dma_start(outr[g], x2[:, g * 4:(g + 1) * 4, :])

    _split_drain_waits(nc)
    return nc


def _run(inputs, trace=False):
    w = _prep_weights(inputs)
    a = float(np.asarray(inputs["alpha"]).reshape(-1)[0])
    nz = {nm: bool(np.any(w[nm])) for nm in
          ("qb", "srb", "kb", "vb", "pjb", "f1b", "f2b")}
    nc = _build_program(a, nz)

    x = np.asarray(inputs["x"], np.float32)
    pos = np.asarray(inputs["pos_2D"], np.float32)
    shared = {k: v for k, v in w.items()
              if k in ("qw", "srw", "kw", "vw", "pjw", "f1w", "dwdp", "dwds",
                       "dwb", "f2w8")}
    for nm in ("qb", "srb", "kb", "vb", "pjb", "f1b", "f2b"):
        if nz[nm]:
            shared[nm] = w[nm]
    in_maps = []
    for b in range(B):
        posT = np.ascontiguousarray(
            pos[b].transpose(0, 2, 1)).astype(ml_dtypes.bfloat16)
        in_maps.append(dict(shared, x=np.ascontiguousarray(x[b]), post=posT))
    res = run_bass_kernel_spmd(nc, in_maps, list(range(B)), trace=trace)
    out = np.stack([res.results[b]["out"] for b in range(B)]).astype(np.float32)
    return out, res


def kernel(**inputs) -> np.ndarray:
    out, _ = _run(inputs, trace=False)
    return out

